# revision 1
# baseline (speedup 1.0000x reference)
"""GAT layer kernel for Trainium2 (Bass/Tile), data-parallel over batch on 8 cores.

v3: all-bf16-pair PE arithmetic, early mask path via s = x @ (W @ w_mlp),
negated-stationary + all-ones-mask trick for the complement sums, batched
transpose copies, pipelined matvec tails.

Per-core computation (batch b, N=2048, F=128):
    p = exp(s), q = exp(0.2 s), M_ij = [s_i + s_j > 0] (bf16 0/1, symmetric)
    D_i   = p_i * (M p)_i + q_i * (sum(q) - (M q)_i)
    col_n = p_n * (M r)_n + q_n * (sum(u) - (M u)_n),  r = p/D, u = q/D
    out   = lrelu(h) * col
All PE stationaries/moving operands are bf16 hi/lo pairs (exact to ~2^-17).
"""

import sys

if "/opt/trn_rl_repo" not in sys.path:
    sys.path.insert(0, "/opt/trn_rl_repo")

from contextlib import ExitStack

import numpy as np

import concourse.bass as bass
import concourse.mybir as mybir
import concourse.tile as tile
from concourse import bacc
from concourse import masks
from concourse.bass_utils import run_bass_kernel_spmd

B, N, F = 8, 2048, 128
NB = N // 128  # 16 token blocks
NC4 = 4  # 512-wide chunks
NEG_SLOPE = 0.2
FP32 = mybir.dt.float32
BF16 = mybir.dt.bfloat16
ALU = mybir.AluOpType
AFT = mybir.ActivationFunctionType


def gat_kernel(ctx: ExitStack, tc: "tile.TileContext", out_d, x_d, W_d, wm_d, bm_d):
    nc = tc.nc

    const_p = ctx.enter_context(tc.tile_pool(name="const", bufs=1))
    big_p = ctx.enter_context(tc.tile_pool(name="big", bufs=1))
    mask_p = ctx.enter_context(tc.tile_pool(name="mask", bufs=NB))
    vec_p = ctx.enter_context(tc.tile_pool(name="vec", bufs=1))
    outsb_p = ctx.enter_context(tc.tile_pool(name="outsb", bufs=4))
    # PSUM: 8 banks total. big=4 banks (hT -> d_ps -> g_ps), tr=2x1 bank
    # (transposes + S_row broadcast chunks), sm=2x1 bank (small matmul outs).
    ps_big = ctx.enter_context(tc.tile_pool(name="ps_big", bufs=1, space="PSUM"))
    ps_tr = ctx.enter_context(tc.tile_pool(name="ps_tr", bufs=3, space="PSUM"))
    ps_sm = ctx.enter_context(tc.tile_pool(name="ps_sm", bufs=1, space="PSUM"))

    # ---------------- input DMAs first (x is the critical path) ----------
    W_sb = const_p.tile([128, 128], FP32, tag="W_sb")
    nc.sync.dma_start(W_sb[:], W_d[:, :])
    wm_sb = const_p.tile([128, 1], FP32, tag="wm_sb")
    nc.scalar.dma_start(wm_sb[:], wm_d.rearrange("(p o) -> p o", o=1))
    b_sb = const_p.tile([1, 1], FP32, tag="b_sb")
    nc.scalar.dma_start(b_sb[:], bm_d.rearrange("(p o) -> p o", o=1))
    x_view = x_d.rearrange("(t p) f -> p t f", p=128)
    x_sb = big_p.tile([128, NB, 128], FP32, tag="x_sb")
    for t in range(NB):
        eng = nc.sync if t % 2 == 0 else nc.scalar
        eng.dma_start(x_sb[:, t, :], x_view[:, t, :])

    # ---------------- constants ----------------
    ident_f = const_p.tile([128, 128], FP32, tag="ident_f")
    ident_b = const_p.tile([128, 128], BF16, tag="ident_b")
    masks.make_identity(nc, ident_f[:])
    masks.make_identity(nc, ident_b[:])
    ones_f = const_p.tile([128, 1], FP32, tag="ones_f")
    nc.gpsimd.memset(ones_f[:], 1.0)
    ones_row_f = const_p.tile([1, 128], FP32, tag="ones_row_f")
    nc.gpsimd.memset(ones_row_f[:], 1.0)
    ones_row_b = const_p.tile([1, 128], BF16, tag="ones_row_b")
    nc.gpsimd.memset(ones_row_b[:], 1.0)

    # Preload the ACT exp table set early (one-time ~2.7us, off critical path)
    warm = const_p.tile([128, 1], FP32, tag="warm")
    nc.scalar.activation(warm[:], ones_f[:], AFT.Exp)

    # all-ones bf16 mask tile for the complement (Qtot/Utot) matvec block
    ones_mask = const_p.tile([128, N], BF16, tag="ones_mask")
    nc.gpsimd.memset(ones_mask[:], 1.0)

    # b broadcast to [128,1] via K=1 PE matmul
    b_ps = ps_sm.tile([128, 1], FP32, tag="sm")
    nc.tensor.matmul(b_ps[:], lhsT=ones_row_f[:], rhs=b_sb[:], start=True, stop=True)
    b_bc = const_p.tile([128, 1], FP32, tag="b_bc")
    nc.vector.tensor_copy(b_bc[:], b_ps[:])

    # ---------------- x -> bf16 hi/lo pair ----------------
    x_hi = big_p.tile([128, NB, 128], BF16, tag="x_hi")
    x_lo = big_p.tile([128, NB, 128], BF16, tag="x_lo")
    for c in range(NC4):
        sl = slice(c * 4, (c + 1) * 4)
        nc.vector.tensor_copy(x_hi[:, sl, :], x_sb[:, sl, :])
        # x_lo = x - fp32(x_hi)  (computed as x_hi * -1 + x on DVE)
        nc.vector.scalar_tensor_tensor(
            x_lo[:, sl, :], x_hi[:, sl, :], -1.0, x_sb[:, sl, :], ALU.mult, ALU.add
        )

    # ---------------- xT pair via PE transposes (bf16) ----------------
    xT_hi = big_p.tile([128, N], BF16, tag="xT_hi")  # [f, tok]
    xT_lo = big_p.tile([128, N], BF16, tag="xT_lo")
    for t2 in range(NB // 2):
        for srct, dst in ((x_hi, xT_hi), (x_lo, xT_lo)):
            tp = ps_tr.tile([128, 256], BF16, tag="trb")
            nc.tensor.matmul(
                tp[:, 0:128], lhsT=srct[:, 2 * t2, :], rhs=ident_b[:],
                is_transpose=True, start=True, stop=False,
            )
            nc.tensor.matmul(
                tp[:, 128:256], lhsT=srct[:, 2 * t2 + 1, :], rhs=ident_b[:],
                is_transpose=True, start=False, stop=True,
            )
            if t2 % 2 == 0:
                nc.vector.tensor_copy(dst[:, t2 * 256 : (t2 + 1) * 256], tp[:])
            else:
                nc.scalar.copy(dst[:, t2 * 256 : (t2 + 1) * 256], tp[:])

    # ---------------- v = W @ w_mlp (via W^T), bf16 pair vk ----------------
    WT_ps = ps_sm.tile([128, 128], FP32, tag="sm")
    nc.tensor.transpose(WT_ps[:], W_sb[:], ident_f[:])
    WT_sb = vec_p.tile([128, 128], FP32, tag="WT_sb")
    nc.vector.tensor_copy(WT_sb[:], WT_ps[:])
    v_ps = ps_sm.tile([128, 1], FP32, tag="sm")
    nc.tensor.matmul(v_ps[:], lhsT=WT_sb[:], rhs=wm_sb[:], start=True, stop=True)
    v_sb = vec_p.tile([128, 1], FP32, tag="v_sb")
    nc.vector.tensor_copy(v_sb[:], v_ps[:])
    vk = vec_p.tile([128, 2], BF16, tag="vk")
    nc.vector.tensor_copy(vk[:, 0:1], v_sb[:])
    v_hi32 = vec_p.tile([128, 1], FP32, tag="v_hi32")
    nc.vector.tensor_copy(v_hi32[:], vk[:, 0:1])
    nc.vector.tensor_tensor(vk[:, 1:2], v_sb[:], v_hi32[:], ALU.subtract)

    # ---------------- s in [128, 16] layout from xT pair ----------------
    # s4[:, t, 0:2] = xT_hi_chunk.T @ [v_hi, v_lo]; s4[:, t, 2] = xT_lo.T @ v_hi
    s4_ps = ps_sm.tile([128, NB, 3], FP32, tag="sm")
    for t in range(NB):
        sl = slice(t * 128, (t + 1) * 128)
        nc.tensor.matmul(
            s4_ps[:, t, 0:2], lhsT=xT_hi[:, sl], rhs=vk[:], start=True, stop=True
        )
        nc.tensor.matmul(
            s4_ps[:, t, 2:3], lhsT=xT_lo[:, sl], rhs=vk[:, 0:1], start=True, stop=True
        )
    s4_sb = vec_p.tile([128, NB, 3], FP32, tag="s4_sb")
    nc.vector.tensor_copy(s4_sb[:], s4_ps[:])
    s12 = vec_p.tile([128, NB], FP32, tag="s12")
    nc.vector.tensor_tensor(s12[:], s4_sb[:, :, 0], s4_sb[:, :, 1], ALU.add)
    s_mat = vec_p.tile([128, NB], FP32, tag="s_mat")
    nc.vector.tensor_tensor(s_mat[:], s12[:], s4_sb[:, :, 2], ALU.add)
    nc.vector.tensor_scalar(s_mat[:], s_mat[:], b_bc[:, 0:1], None, ALU.add)
    neg_s = vec_p.tile([128, NB], FP32, tag="neg_s")
    nc.vector.tensor_scalar(neg_s[:], s_mat[:], -1.0, None, ALU.mult)
    s_hi = vec_p.tile([128, NB], BF16, tag="s_hi")
    nc.vector.tensor_copy(s_hi[:], s_mat[:])

    # p = exp(s), q = exp(0.2 s), hi/lo bf16 splits, packed stationary Pk
    p_v = vec_p.tile([128, NB], FP32, tag="p_v")
    nc.scalar.activation(p_v[:], s_mat[:], AFT.Exp)
    q_v = vec_p.tile([128, NB], FP32, tag="q_v")
    nc.scalar.activation(q_v[:], s_mat[:], AFT.Exp, scale=NEG_SLOPE)

    def hi_lo(src, tagbase):
        hi = vec_p.tile([128, NB], BF16, tag=tagbase + "_hi")
        nc.vector.tensor_copy(hi[:], src[:])
        hi32 = vec_p.tile([128, NB], FP32, tag=tagbase + "_hi32")
        nc.vector.tensor_copy(hi32[:], hi[:])
        lo = vec_p.tile([128, NB], BF16, tag=tagbase + "_lo")
        nc.vector.tensor_tensor(lo[:], src[:], hi32[:], ALU.subtract)
        return hi, lo

    # Pk rows: [p_hi, p_lo, -q_hi, -q_lo]; the all-ones 17th block carries
    # [0, 0, qs_hi, qs_lo] so rows 2-3 accumulate  sum(q) - (M q)  directly.
    Pk = vec_p.tile([128, NB, 4], BF16, tag="Pk")
    nc.vector.tensor_copy(Pk[:, :, 0], p_v[:])
    p_hi32 = vec_p.tile([128, NB], FP32, tag="p_hi32")
    nc.vector.tensor_copy(p_hi32[:], Pk[:, :, 0])
    nc.vector.tensor_tensor(Pk[:, :, 1], p_v[:], p_hi32[:], ALU.subtract)
    nc.vector.tensor_scalar(Pk[:, :, 2], q_v[:], -1.0, None, ALU.mult)
    qn_hi32 = vec_p.tile([128, NB], FP32, tag="qn_hi32")
    nc.vector.tensor_copy(qn_hi32[:], Pk[:, :, 2])
    nc.vector.scalar_tensor_tensor(
        Pk[:, :, 3], qn_hi32[:], -1.0, q_v[:], ALU.mult, ALU.subtract
    )
    # ones-block stationary: per-partition row-sums of q (sum to Qtot exactly)
    qs = vec_p.tile([128, 1], FP32, tag="qs")
    nc.vector.reduce_sum(qs[:], q_v[:], axis=mybir.AxisListType.X)
    qs_hi = vec_p.tile([128, 1], BF16, tag="qs_hi")
    nc.vector.tensor_copy(qs_hi[:], qs[:])
    qs_hi32 = vec_p.tile([128, 1], FP32, tag="qs_hi32")
    nc.vector.tensor_copy(qs_hi32[:], qs_hi[:])
    Pk1 = vec_p.tile([128, 4], BF16, tag="Pk1")
    nc.gpsimd.memset(Pk1[:], 0.0)
    nc.vector.tensor_copy(Pk1[:, 2:3], qs_hi[:])
    nc.vector.tensor_tensor(Pk1[:, 3:4], qs[:], qs_hi32[:], ALU.subtract)

    # ---------------- S_row broadcast [128, 2048] bf16 ----------------
    sT_ps = ps_sm.tile([16, 128], BF16, tag="sm")
    nc.tensor.transpose(sT_ps[:], s_hi[:], ident_b[:])
    sT_sb = vec_p.tile([16, 128], BF16, tag="sT_sb")
    nc.vector.tensor_copy(sT_sb[:], sT_ps[:])
    s_flat = vec_p.tile([1, N], BF16, tag="s_flat")
    nc.sync.dma_start(s_flat[0:1, :], sT_sb[:, :])
    S_row = big_p.tile([128, N], BF16, tag="S_row")
    for c in range(NC4):
        sl = slice(c * 512, (c + 1) * 512)
        S_ps = ps_tr.tile([128, 512], FP32, tag="trb")
        nc.tensor.matmul(
            S_ps[:], lhsT=ones_row_b[:], rhs=s_flat[0:1, sl], start=True, stop=True
        )
        nc.scalar.copy(S_row[:, sl], S_ps[:])

    # ---------------- hT (bf16 pairs) + lrelu + out transposes ------------
    # emitted after matvec-1 in program order, but depends only on xT/W:
    # Tile schedules it into PE idle windows (mask wait, D tail).
    W_hi = const_p.tile([128, 128], BF16, tag="W_hi")
    nc.scalar.copy(W_hi[:], W_sb[:])
    W_hi32 = const_p.tile([128, 128], FP32, tag="W_hi32")
    nc.scalar.copy(W_hi32[:], W_hi[:])
    W_lo = const_p.tile([128, 128], BF16, tag="W_lo")
    nc.vector.tensor_tensor(W_lo[:], W_sb[:], W_hi32[:], ALU.subtract)

    hT_ps = ps_big.tile([128, N], FP32, tag="bigps")
    for c in range(NC4):
        sl = slice(c * 512, (c + 1) * 512)
        nc.tensor.matmul(
            hT_ps[:, sl], lhsT=W_hi[:], rhs=xT_hi[:, sl], start=True, stop=False
        )
        nc.tensor.matmul(
            hT_ps[:, sl], lhsT=W_hi[:], rhs=xT_lo[:, sl], start=False, stop=False
        )
        nc.tensor.matmul(
            hT_ps[:, sl], lhsT=W_lo[:], rhs=xT_hi[:, sl], start=False, stop=True
        )
    hT_sb = big_p.tile([128, N], FP32, tag="hT_sb")
    for c in range(NC4):
        sl = slice(c * 512, (c + 1) * 512)
        nc.scalar.copy(hT_sb[:, sl], hT_ps[:, sl])

    # ---------------- masks M_a = [s_j > -s_i], bf16 0/1 ----------------
    mask_tiles = []
    for a in range(NB):
        m = mask_p.tile([128, N], BF16, tag="mask")
        for c in range(NC4):
            sl = slice(c * 512, (c + 1) * 512)
            nc.vector.tensor_scalar(
                m[:, sl], S_row[:, sl], neg_s[:, a : a + 1], None, ALU.is_gt
            )
        mask_tiles.append(m)

    # ---------------- matvec 1 (a-outer: pipelines with mask build) -------
    d_ps = ps_big.tile([4, N], FP32, tag="bigps")
    for a in range(NB + 1):
        lhsT = Pk1[:] if a == NB else Pk[:, a, :]
        rhs_t = ones_mask if a == NB else mask_tiles[a]
        for c in range(NC4):
            nc.tensor.matmul(
                d_ps[:, c * 512 : (c + 1) * 512],
                lhsT=lhsT,
                rhs=rhs_t[:, c * 512 : (c + 1) * 512],
                start=(a == 0),
                stop=(a == NB),
            )

    # lrelu(hT) then transpose tiles to [tok, f] layout in SBUF (pre-staged)
    lrlT = big_p.tile([128, N], FP32, tag="lrlT")
    for c in range(NC4):
        sl = slice(c * 512, (c + 1) * 512)
        nc.vector.scalar_tensor_tensor(
            lrlT[:, sl], hT_sb[:, sl], NEG_SLOPE, hT_sb[:, sl], ALU.mult, ALU.max
        )
    lrl_sb = big_p.tile([128, NB, 128], FP32, tag="lrl_sb")
    for t2 in range(NB // 2):
        op = ps_tr.tile([128, 256], FP32, tag="trb")
        nc.tensor.matmul(
            op[:, 0:128], lhsT=lrlT[:, 256 * t2 : 256 * t2 + 128], rhs=ident_f[:],
            is_transpose=True, start=True, stop=False,
        )
        nc.tensor.matmul(
            op[:, 128:256], lhsT=lrlT[:, 256 * t2 + 128 : 256 * t2 + 256], rhs=ident_f[:],
            is_transpose=True, start=False, stop=True,
        )
        nc.scalar.copy(lrl_sb[:, 2 * t2 : 2 * t2 + 2, :], op[:])



    # ---------------- D tail: transpose + combine -------------------------
    d_sb = vec_p.tile([4, N], FP32, tag="d_sb")
    for c in range(NC4):
        sl = slice(c * 512, (c + 1) * 512)
        if c % 2 == 0:
            nc.vector.tensor_copy(d_sb[:, sl], d_ps[:, sl])
        else:
            nc.scalar.copy(d_sb[:, sl], d_ps[:, sl])
    Dp = vec_p.tile([128, NB, 4], FP32, tag="Dp")
    for c in range(NC4):
        dtp = ps_tr.tile([128, 16], FP32, tag="trb")
        for tt in range(4):
            t = c * 4 + tt
            nc.tensor.matmul(
                dtp[:, 4 * tt : 4 * tt + 4],
                lhsT=d_sb[:, t * 128 : (t + 1) * 128],
                rhs=ident_f[0:4, 0:4],
                is_transpose=True, start=(tt == 0), stop=(tt == 3),
            )
        if c % 2 == 0:
            nc.vector.tensor_copy(Dp[:, c * 4 : c * 4 + 4, :], dtp[:])
        else:
            nc.scalar.copy(Dp[:, c * 4 : c * 4 + 4, :], dtp[:])

    # D = p*(d0+d1) + q*(d2+d3)   (rows 2-3 already hold Qtot - M q)
    A_v = vec_p.tile([128, NB], FP32, tag="A_v")
    nc.vector.tensor_tensor(A_v[:], Dp[:, :, 0], Dp[:, :, 1], ALU.add)
    MQ = vec_p.tile([128, NB], FP32, tag="MQ")
    nc.vector.tensor_tensor(MQ[:], Dp[:, :, 2], Dp[:, :, 3], ALU.add)
    t1 = vec_p.tile([128, NB], FP32, tag="t1")
    nc.vector.tensor_tensor(t1[:], p_v[:], A_v[:], ALU.mult)
    t2 = vec_p.tile([128, NB], FP32, tag="t2")
    nc.vector.tensor_tensor(t2[:], q_v[:], MQ[:], ALU.mult)
    D_v = vec_p.tile([128, NB], FP32, tag="D_v")
    nc.vector.tensor_tensor(D_v[:], t1[:], t2[:], ALU.add)
    invD = vec_p.tile([128, NB], FP32, tag="invD")
    nc.vector.reciprocal(invD[:], D_v[:])
    r_v = vec_p.tile([128, NB], FP32, tag="r_v")
    nc.vector.tensor_tensor(r_v[:], p_v[:], invD[:], ALU.mult)
    u_v = vec_p.tile([128, NB], FP32, tag="u_v")
    nc.vector.tensor_tensor(u_v[:], q_v[:], invD[:], ALU.mult)
    Rk = vec_p.tile([128, NB, 4], BF16, tag="Rk")
    nc.vector.tensor_copy(Rk[:, :, 0], r_v[:])
    r_hi32 = vec_p.tile([128, NB], FP32, tag="r_hi32")
    nc.vector.tensor_copy(r_hi32[:], Rk[:, :, 0])
    nc.vector.tensor_tensor(Rk[:, :, 1], r_v[:], r_hi32[:], ALU.subtract)
    nc.vector.tensor_scalar(Rk[:, :, 2], u_v[:], -1.0, None, ALU.mult)
    un_hi32 = vec_p.tile([128, NB], FP32, tag="un_hi32")
    nc.vector.tensor_copy(un_hi32[:], Rk[:, :, 2])
    nc.vector.scalar_tensor_tensor(
        Rk[:, :, 3], un_hi32[:], -1.0, u_v[:], ALU.mult, ALU.subtract
    )
    us = vec_p.tile([128, 1], FP32, tag="us")
    nc.vector.reduce_sum(us[:], u_v[:], axis=mybir.AxisListType.X)
    us_hi = vec_p.tile([128, 1], BF16, tag="us_hi")
    nc.vector.tensor_copy(us_hi[:], us[:])
    us_hi32 = vec_p.tile([128, 1], FP32, tag="us_hi32")
    nc.vector.tensor_copy(us_hi32[:], us_hi[:])
    Rk1 = vec_p.tile([128, 4], BF16, tag="Rk1")
    nc.gpsimd.memset(Rk1[:], 0.0)
    nc.vector.tensor_copy(Rk1[:, 2:3], us_hi[:])
    nc.vector.tensor_tensor(Rk1[:, 3:4], us[:], us_hi32[:], ALU.subtract)

    # ---------------- matvec 2 (c-outer: per-chunk output tails) ----------
    out_view = out_d.rearrange("(t p) f -> p t f", p=128)
    g_sb = vec_p.tile([4, N], FP32, tag="g_sb")
    Gp = vec_p.tile([128, NB, 4], FP32, tag="Gp")
    col = vec_p.tile([128, NB], FP32, tag="col")
    gl = vec_p.tile([128, NB], FP32, tag="gl")
    gu2 = vec_p.tile([128, NB], FP32, tag="gu2")
    def mv2_group(c):
        sl = slice(c * 512, (c + 1) * 512)
        g_ps = ps_tr.tile([4, 512], FP32, tag="trb")
        for a in range(NB + 1):
            nc.tensor.matmul(
                g_ps[:],
                lhsT=Rk1[:] if a == NB else Rk[:, a, :],
                rhs=(ones_mask if a == NB else mask_tiles[a])[:, sl],
                start=(a == 0),
                stop=(a == NB),
            )
        return g_ps

    def mv2_tail(c, g_ps):
        sl = slice(c * 512, (c + 1) * 512)
        if c % 2 == 0:
            nc.vector.tensor_copy(g_sb[:, sl], g_ps[:])
        else:
            nc.scalar.copy(g_sb[:, sl], g_ps[:])
        gtp = ps_tr.tile([128, 16], FP32, tag="trb")
        for tt in range(4):
            t = c * 4 + tt
            nc.tensor.matmul(
                gtp[:, 4 * tt : 4 * tt + 4],
                lhsT=g_sb[:, t * 128 : (t + 1) * 128],
                rhs=ident_f[0:4, 0:4],
                is_transpose=True, start=(tt == 0), stop=(tt == 3),
            )
        tsl = slice(c * 4, (c + 1) * 4)
        if c % 2 == 0:
            nc.vector.tensor_copy(Gp[:, tsl, :], gtp[:])
        else:
            nc.scalar.copy(Gp[:, tsl, :], gtp[:])
        # col = p*(G0+G1) + q*(G2+G3)   (rows 2-3 already hold Utot - M u)
        nc.vector.tensor_tensor(gl[:, tsl], Gp[:, tsl, 0], Gp[:, tsl, 1], ALU.add)
        nc.vector.tensor_tensor(gu2[:, tsl], Gp[:, tsl, 2], Gp[:, tsl, 3], ALU.add)
        nc.vector.tensor_tensor(gl[:, tsl], p_v[:, tsl], gl[:, tsl], ALU.mult)
        nc.vector.tensor_tensor(gu2[:, tsl], q_v[:, tsl], gu2[:, tsl], ALU.mult)
        nc.vector.tensor_tensor(col[:, tsl], gl[:, tsl], gu2[:, tsl], ALU.add)
        for tt in range(4):
            t = c * 4 + tt
            o_sb = outsb_p.tile([128, 128], FP32, tag="o_sb")
            if t % 2 == 0:
                nc.vector.tensor_scalar(
                    o_sb[:], lrl_sb[:, t, :], col[:, t : t + 1], None, ALU.mult
                )
            else:
                nc.scalar.activation(
                    o_sb[:], lrl_sb[:, t, :], AFT.Copy, scale=col[:, t : t + 1]
                )
            (nc.sync if t % 2 == 0 else nc.scalar).dma_start(
                out_view[:, t, :], o_sb[:]
            )

    pending = None
    for c in range(NC4):
        g_ps_c = mv2_group(c)
        if pending is not None:
            mv2_tail(pending[0], pending[1])
        pending = (c, g_ps_c)
    mv2_tail(pending[0], pending[1])

def build_nc(num_devices: int = 8) -> "bass.Bass":
    nc = bacc.Bacc(
        "TRN2", target_bir_lowering=False, debug=False, num_devices=num_devices
    )
    x_d = nc.dram_tensor("x", [N, F], FP32, kind="ExternalInput")
    W_d = nc.dram_tensor("W", [F, F], FP32, kind="ExternalInput")
    wm_d = nc.dram_tensor("w_mlp", [F], FP32, kind="ExternalInput")
    bm_d = nc.dram_tensor("b_mlp", [1], FP32, kind="ExternalInput")
    out_d = nc.dram_tensor("out", [N, F], FP32, kind="ExternalOutput")
    with tile.TileContext(nc) as tc:
        with ExitStack() as ctx:
            gat_kernel(ctx, tc, out_d.ap(), x_d.ap(), W_d.ap(), wm_d.ap(), bm_d.ap())
    nc.compile()
    return nc


_NC_CACHE: dict = {}


def run(x, W, w_mlp, b_mlp, trace=False, **spmd_kwargs):
    x = np.asarray(x, dtype=np.float32)
    W = np.asarray(W, dtype=np.float32)
    w_mlp = np.asarray(w_mlp, dtype=np.float32)
    b_mlp = np.asarray(b_mlp, dtype=np.float32)

    if "nc" not in _NC_CACHE:
        _NC_CACHE["nc"] = build_nc(num_devices=B)
    nc = _NC_CACHE["nc"]

    in_maps = [
        {"x": np.ascontiguousarray(x[b, 0]), "W": W, "w_mlp": w_mlp, "b_mlp": b_mlp}
        for b in range(B)
    ]
    res = run_bass_kernel_spmd(
        nc, in_maps, core_ids=list(range(B)), trace=trace, **spmd_kwargs
    )
    out = np.stack([res.results[b]["out"] for b in range(B)])[:, None]
    return out.astype(np.float32), res


def kernel(x, W, w_mlp, b_mlp):
    out, _ = run(x, W, w_mlp, b_mlp)
    return out



# revision 2
# speedup vs baseline: 1.0196x; 1.0196x over previous
"""GAT layer kernel for Trainium2 (Bass/Tile), data-parallel over batch on 8 cores.

v7: bf16 masks + bf16 single stationaries (fp8 on DVE/Pool hits a slow
custom-uop path; ACT fp8 ok but PE win not worth it). Mask build split
DVE is_gt (0/1) + ACT Sign (+-1, halved stationaries + folded constants).
Single-bf16 x/W, S_row via V128 broadcast matmul (b folded into exp bias
and mask thresholds), hT+lrelu in the D-tail PE gap, multi-queue x DMA.

Math (per core, N=2048):
    s' = x@W@w_mlp (no bias);  p = exp(s'+b), q = exp(0.2(s'+b))
    M_ij = [s'_i + s'_j + 2b > 0]
    D   = p*(Mp) + q*(Qtot - Mq)
    col = p*(Mr) + q*(Utot - Mu),  r = p/D, u = q/D  (M symmetric)
    out = lrelu(h) * col,  h = x@W
Sign-built blocks use t=+-1 masks with stationary p/2 so PSUM accumulates
(Mp) - P_S/2; the constants fold into the combine tails.
"""

import sys

if "/opt/trn_rl_repo" not in sys.path:
    sys.path.insert(0, "/opt/trn_rl_repo")

from contextlib import ExitStack

import numpy as np

import concourse.bass as bass
import concourse.mybir as mybir
import concourse.tile as tile
from concourse import bacc
from concourse import masks
from concourse.bass_utils import run_bass_kernel_spmd

B, N, F = 8, 2048, 128
NB = N // 128  # 16 token blocks
NC4 = 4  # 512-wide chunks
NEG_SLOPE = 0.2
FP32 = mybir.dt.float32
BF16 = mybir.dt.bfloat16
ALU = mybir.AluOpType
AFT = mybir.ActivationFunctionType

# mask build engine per block: D=vector is_gt 0/1, A=scalar Sign +-1
BLK_ENG = ["D", "D", "D", "A", "D", "D", "D", "A",
           "D", "D", "D", "A", "D", "D", "D", "A"]
A_BLOCKS = [3, 7, 11, 15]  # stride-4 for slicing


def gat_kernel(ctx: ExitStack, tc: "tile.TileContext", out_d, x_d, W_d, wm_d, bm_d):
    nc = tc.nc

    const_p = ctx.enter_context(tc.tile_pool(name="const", bufs=1))
    big_p = ctx.enter_context(tc.tile_pool(name="big", bufs=1))
    mask_p = ctx.enter_context(tc.tile_pool(name="mask", bufs=NB))
    vec_p = ctx.enter_context(tc.tile_pool(name="vec", bufs=1))
    outsb_p = ctx.enter_context(tc.tile_pool(name="outsb", bufs=4))
    # PSUM banks: big=4 (hT / d_ps, sequential), trb=2x1, sm=1
    ps_big = ctx.enter_context(tc.tile_pool(name="ps_big", bufs=1, space="PSUM"))
    ps_tr = ctx.enter_context(tc.tile_pool(name="ps_tr", bufs=3, space="PSUM"))
    ps_sm = ctx.enter_context(tc.tile_pool(name="ps_sm", bufs=1, space="PSUM"))

    # ---------------- input DMAs first (x is the critical path) ----------
    W_sb = const_p.tile([128, 128], FP32, tag="W_sb")
    nc.sync.dma_start(W_sb[:], W_d[:, :])
    wm_sb = const_p.tile([128, 1], FP32, tag="wm_sb")
    nc.scalar.dma_start(wm_sb[:], wm_d.rearrange("(p o) -> p o", o=1))
    b_sb = const_p.tile([1, 1], FP32, tag="b_sb")
    nc.scalar.dma_start(b_sb[:], bm_d.rearrange("(p o) -> p o", o=1))
    x_view = x_d.rearrange("(t p) f -> p t f", p=128)
    x_sb = big_p.tile([128, NB, 128], FP32, tag="x_sb")
    dma_engs = [nc.sync, nc.scalar, nc.gpsimd]
    for t2 in range(8):
        eng = dma_engs[t2 % 3]
        eng.dma_start(x_sb[:, 2 * t2 : 2 * t2 + 2, :], x_view[:, 2 * t2 : 2 * t2 + 2, :])

    # ---------------- constants ----------------
    ident_f = const_p.tile([128, 128], FP32, tag="ident_f")
    ident_b = const_p.tile([128, 128], BF16, tag="ident_b")
    masks.make_identity(nc, ident_f[:])
    masks.make_identity(nc, ident_b[:])
    ones_row_f = const_p.tile([1, 128], FP32, tag="ones_row_f")
    nc.gpsimd.memset(ones_row_f[:], 1.0)
    ones_f128 = const_p.tile([128, 128], FP32, tag="ones_f128")
    nc.gpsimd.memset(ones_f128[:], 1.0)
    ones_col = const_p.tile([128, 1], FP32, tag="ones_col")
    nc.gpsimd.memset(ones_col[:], 1.0)

    # Preload the ACT exp table set early (Sign/Copy/Exp share one set)
    warm = const_p.tile([128, 1], FP32, tag="warm")
    nc.scalar.activation(warm[:], ones_col[:], AFT.Exp)

    # b broadcast to [128,1] via K=1 PE matmul
    b_ps = ps_sm.tile([128, 1], FP32, tag="sm")
    nc.tensor.matmul(b_ps[:], lhsT=ones_row_f[:], rhs=b_sb[:], start=True, stop=True)
    b_bc = const_p.tile([128, 1], FP32, tag="b_bc")
    nc.vector.tensor_copy(b_bc[:], b_ps[:])
    b02 = const_p.tile([128, 1], FP32, tag="b02")
    nc.vector.tensor_scalar(b02[:], b_bc[:], NEG_SLOPE, None, ALU.mult)

    # ---------------- x -> bf16, xT via PE transposes ----------------
    x_hi = big_p.tile([128, NB, 128], BF16, tag="x_hi")
    for t2 in range(8):
        sl2 = slice(2 * t2, 2 * t2 + 2)
        if t2 in (5, 7):
            nc.scalar.copy(x_hi[:, sl2, :], x_sb[:, sl2, :])
        else:
            nc.vector.tensor_copy(x_hi[:, sl2, :], x_sb[:, sl2, :])

    xT = big_p.tile([128, N], BF16, tag="xT")  # [f, tok]
    xT_cp = [nc.vector, nc.scalar, nc.vector, nc.scalar,
             nc.vector, nc.scalar, nc.vector, nc.scalar]
    for t2 in range(8):
        tp = ps_tr.tile([128, 256], BF16, tag="trb")
        nc.tensor.matmul(
            tp[:, 0:128], lhsT=x_hi[:, 2 * t2, :], rhs=ident_b[:],
            is_transpose=True, start=True, stop=False,
        )
        nc.tensor.matmul(
            tp[:, 128:256], lhsT=x_hi[:, 2 * t2 + 1, :], rhs=ident_b[:],
            is_transpose=True, start=False, stop=True,
        )
        e = xT_cp[t2]
        if e is nc.scalar:
            e.copy(xT[:, t2 * 256 : (t2 + 1) * 256], tp[:])
        else:
            e.tensor_copy(xT[:, t2 * 256 : (t2 + 1) * 256], tp[:])

    # ---------------- W chain: v = W @ w_mlp, V128, W_hi ----------------
    WT_ps = ps_sm.tile([128, 128], FP32, tag="sm")
    nc.tensor.transpose(WT_ps[:], W_sb[:], ident_f[:])
    WT_sb = vec_p.tile([128, 128], FP32, tag="WT_sb")
    nc.vector.tensor_copy(WT_sb[:], WT_ps[:])
    v_ps = ps_sm.tile([128, 1], FP32, tag="sm")
    nc.tensor.matmul(v_ps[:], lhsT=WT_sb[:], rhs=wm_sb[:], start=True, stop=True)
    v_sb = vec_p.tile([128, 1], FP32, tag="v_sb")
    nc.vector.tensor_copy(v_sb[:], v_ps[:])
    vk = vec_p.tile([128, 1], BF16, tag="vk")
    nc.vector.tensor_copy(vk[:], v_sb[:])
    W_hi = const_p.tile([128, 128], BF16, tag="W_hi")
    nc.scalar.copy(W_hi[:], W_sb[:])
    # V128[:, c] = v for all c (for S_row broadcast matmul)
    V128 = vec_p.tile([128, 128], BF16, tag="V128")
    nc.vector.tensor_scalar(V128[:], ones_f128[:], v_sb[:, 0:1], None, ALU.mult)

    # ---------------- s' [128, 16] via 16 small matmuls (no bias) --------
    s_ps = ps_sm.tile([128, NB], FP32, tag="sm")
    for t in range(NB):
        nc.tensor.matmul(
            s_ps[:, t : t + 1], lhsT=xT[:, t * 128 : (t + 1) * 128], rhs=vk[:],
            start=True, stop=True,
        )
    s_mat = vec_p.tile([128, NB], FP32, tag="s_mat")
    nc.vector.tensor_copy(s_mat[:], s_ps[:])
    # thresholds: is_gt blocks use -s'_a - 2b; Sign blocks bias s'_a + 2b
    neg_s = vec_p.tile([128, NB], FP32, tag="neg_s")
    nc.vector.tensor_scalar(neg_s[:], s_mat[:], -1.0, None, ALU.mult)
    nc.vector.tensor_scalar(neg_s[:], neg_s[:], b_bc[:, 0:1], None, ALU.subtract)
    nc.vector.tensor_scalar(neg_s[:], neg_s[:], b_bc[:, 0:1], None, ALU.subtract)
    pos_s2b = vec_p.tile([128, NB], FP32, tag="pos_s2b")
    nc.vector.tensor_scalar(pos_s2b[:], s_mat[:], b_bc[:, 0:1], None, ALU.add)
    nc.vector.tensor_scalar(pos_s2b[:], pos_s2b[:], b_bc[:, 0:1], None, ALU.add)

    # ---------------- S_row [128, 2048] bf16 = s'_n broadcast ------------
    S_row = big_p.tile([128, N], BF16, tag="S_row")
    s_cp = [nc.scalar, nc.vector, nc.scalar, nc.vector]
    for c in range(NC4):
        sl = slice(c * 512, (c + 1) * 512)
        S_ps = ps_tr.tile([128, 512], FP32, tag="trb")
        nc.tensor.matmul(S_ps[:], lhsT=V128[:], rhs=xT[:, sl], start=True, stop=True)
        e = s_cp[c]
        if e is nc.scalar:
            e.copy(S_row[:, sl], S_ps[:])
        else:
            e.tensor_copy(S_row[:, sl], S_ps[:])

    # ---------------- p, q, bf16 stationaries, constants ----------------
    p_v = vec_p.tile([128, NB], FP32, tag="p_v")
    nc.scalar.activation(p_v[:], s_mat[:], AFT.Exp, bias=b_bc[:, 0:1])
    q_v = vec_p.tile([128, NB], FP32, tag="q_v")
    nc.scalar.activation(q_v[:], s_mat[:], AFT.Exp, scale=NEG_SLOPE, bias=b02[:, 0:1])

    def make_stationary(pv, qv, tagbase):
        """bf16 [128, NB, 2] rows (pv, -qv); A-blocks halved (sign masks)."""
        K = vec_p.tile([128, NB, 2], BF16, tag=tagbase)
        nc.vector.tensor_copy(K[:, :, 0], pv[:])
        nc.vector.tensor_scalar(K[:, :, 1], qv[:], -1.0, None, ALU.mult)
        nc.vector.tensor_scalar(K[:, 3::4, 0], pv[:, 3::4], 0.5, None, ALU.mult)
        nc.vector.tensor_scalar(K[:, 3::4, 1], qv[:, 3::4], -0.5, None, ALU.mult)
        return K

    def make_consts(pv, qv, tagbase):
        """c_p = P_A/2 bcast, c_q = TOT_q - Q_A/2 bcast  ([128,1] each)."""
        cin = vec_p.tile([128, 3], FP32, tag=tagbase + "_in")
        nc.vector.reduce_sum(cin[:, 0:1], qv[:], axis=mybir.AxisListType.X)
        nc.vector.reduce_sum(cin[:, 1:2], pv[:, 3::4], axis=mybir.AxisListType.X)
        nc.vector.reduce_sum(cin[:, 2:3], qv[:, 3::4], axis=mybir.AxisListType.X)
        cps = ps_sm.tile([128, 3], FP32, tag="sm")
        nc.tensor.matmul(cps[:], lhsT=ones_f128[:], rhs=cin[:], start=True, stop=True)
        cbc = vec_p.tile([128, 3], FP32, tag=tagbase + "_bc")
        nc.vector.tensor_copy(cbc[:], cps[:])
        c_p = vec_p.tile([128, 1], FP32, tag=tagbase + "_cp")
        nc.vector.tensor_scalar(c_p[:], cbc[:, 1:2], 0.5, None, ALU.mult)
        c_q = vec_p.tile([128, 1], FP32, tag=tagbase + "_cq")
        nc.vector.tensor_scalar(c_q[:], cbc[:, 2:3], -0.5, None, ALU.mult)
        nc.vector.tensor_tensor(c_q[:], c_q[:], cbc[:, 0:1], ALU.add)
        return c_p, c_q

    Pk = make_stationary(p_v, q_v, "Pk")
    c1, c2 = make_consts(p_v, q_v, "k1")

    # ---------------- masks: bf16, DVE is_gt + ACT Sign ----------------
    m_tiles = []
    for _a in range(NB):
        m_t = mask_p.tile([128, N], BF16, tag="mask")
        m_tiles.append(m_t)
    for a in range(NB):
        for c in range(NC4):
            sl = slice(c * 512, (c + 1) * 512)
            if BLK_ENG[a] == "A":
                nc.scalar.activation(
                    m_tiles[a][:, sl], S_row[:, sl], AFT.Sign,
                    bias=pos_s2b[:, a : a + 1],
                )
            else:
                nc.vector.tensor_scalar(
                    m_tiles[a][:, sl], S_row[:, sl], neg_s[:, a : a + 1], None,
                    ALU.is_gt,
                )

    # ---------------- matvec 1 (block-outer) ----------------
    d_ps = ps_big.tile([2, N], FP32, tag="bigps")
    for a in range(NB):
        for c in range(NC4):
            nc.tensor.matmul(
                d_ps[:, c * 512 : (c + 1) * 512],
                lhsT=Pk[:, a, :],
                rhs=m_tiles[a][:, c * 512 : (c + 1) * 512],
                start=(a == 0),
                stop=(a == NB - 1),
            )

    # ---------------- D tail: transpose + combine ----------------
    d_sb = vec_p.tile([2, N], FP32, tag="d_sb")
    d_cp = [nc.vector, nc.scalar, nc.vector, nc.scalar]
    for c in range(NC4):
        sl = slice(c * 512, (c + 1) * 512)
        e = d_cp[c]
        if e is nc.scalar:
            e.copy(d_sb[:, sl], d_ps[:, sl])
        else:
            e.tensor_copy(d_sb[:, sl], d_ps[:, sl])
    Dp = vec_p.tile([128, NB, 2], FP32, tag="Dp")
    for c in range(NC4):
        dtp = ps_tr.tile([128, 8], FP32, tag="trb")
        for t4 in range(4):
            t = c * 4 + t4
            nc.tensor.matmul(
                dtp[:, 2 * t4 : 2 * t4 + 2],
                lhsT=d_sb[:, t * 128 : (t + 1) * 128],
                rhs=ident_f[0:2, 0:2],
                is_transpose=True, start=(t4 == 0), stop=(t4 == 3),
            )
        if c % 2 == 0:
            nc.vector.tensor_copy(Dp[:, c * 4 : c * 4 + 4, :], dtp[:])
        else:
            nc.scalar.copy(Dp[:, c * 4 : c * 4 + 4, :], dtp[:])

    # D = p*(row0 + c1) + q*(row1 + c2);  r = p/D, u = q/D
    A1 = vec_p.tile([128, NB], FP32, tag="A1")
    nc.vector.tensor_scalar(A1[:], Dp[:, :, 0], c1[:, 0:1], None, ALU.add)
    B1 = vec_p.tile([128, NB], FP32, tag="B1")
    nc.vector.tensor_scalar(B1[:], Dp[:, :, 1], c2[:, 0:1], None, ALU.add)
    t1 = vec_p.tile([128, NB], FP32, tag="t1")
    nc.vector.tensor_tensor(t1[:], p_v[:], A1[:], ALU.mult)
    t2 = vec_p.tile([128, NB], FP32, tag="t2")
    nc.vector.tensor_tensor(t2[:], q_v[:], B1[:], ALU.mult)
    D_v = vec_p.tile([128, NB], FP32, tag="D_v")
    nc.vector.tensor_tensor(D_v[:], t1[:], t2[:], ALU.add)
    invD = vec_p.tile([128, NB], FP32, tag="invD")
    nc.vector.reciprocal(invD[:], D_v[:])
    rr = vec_p.tile([128, NB], FP32, tag="rr")
    nc.vector.tensor_tensor(rr[:], p_v[:], invD[:], ALU.mult)
    uu = vec_p.tile([128, NB], FP32, tag="uu")
    nc.vector.tensor_tensor(uu[:], q_v[:], invD[:], ALU.mult)
    Rk = make_stationary(rr, uu, "Rk")
    c3, c4 = make_consts(rr, uu, "k2")

    # ---------------- hT + lrelu (PE gap during D tail) ----------------
    # lrelu(x) = 0.8*relu(x) + 0.2*x: ACT produces both pieces from PSUM
    # (emitted after its Sign blocks), DVE adds them post-Rk.
    hT_ps = ps_big.tile([128, N], FP32, tag="bigps")
    for c in range(NC4):
        sl = slice(c * 512, (c + 1) * 512)
        nc.tensor.matmul(hT_ps[:, sl], lhsT=W_hi[:], rhs=xT[:, sl], start=True, stop=True)
    rel08 = big_p.tile([128, N], BF16, tag="rel08")
    lin02 = big_p.tile([128, N], BF16, tag="lin02")
    for c in range(NC4):
        sl = slice(c * 512, (c + 1) * 512)
        nc.scalar.activation(rel08[:, sl], hT_ps[:, sl], AFT.Relu, scale=0.8)
        nc.scalar.activation(lin02[:, sl], hT_ps[:, sl], AFT.Identity, scale=0.2)
    lrlT = big_p.tile([128, N], BF16, tag="lrlT")
    for c in range(NC4):
        sl = slice(c * 512, (c + 1) * 512)
        nc.vector.tensor_tensor(lrlT[:, sl], rel08[:, sl], lin02[:, sl], ALU.add)
    lrl_sb = big_p.tile([128, NB, 128], BF16, tag="lrl_sb")
    lrl_cp = [nc.vector, nc.scalar, nc.vector, nc.scalar,
              nc.vector, nc.scalar, nc.vector, nc.scalar]

    def emit_lrl_transposes():
        for t2 in range(8):
            op = ps_tr.tile([128, 256], BF16, tag="trb")
            nc.tensor.matmul(
                op[:, 0:128], lhsT=lrlT[:, 256 * t2 : 256 * t2 + 128], rhs=ident_b[:],
                is_transpose=True, start=True, stop=False,
            )
            nc.tensor.matmul(
                op[:, 128:256], lhsT=lrlT[:, 256 * t2 + 128 : 256 * t2 + 256],
                rhs=ident_b[:], is_transpose=True, start=False, stop=True,
            )
            e = lrl_cp[t2]
            if e is nc.scalar:
                e.copy(lrl_sb[:, 2 * t2 : 2 * t2 + 2, :], op[:])
            else:
                e.tensor_copy(lrl_sb[:, 2 * t2 : 2 * t2 + 2, :], op[:])

    # ---------------- matvec 2 (c-outer) + pipelined tails ----------
    out_view = out_d.rearrange("(t p) f -> p t f", p=128)
    g_sb = vec_p.tile([2, N], FP32, tag="g_sb")
    Gp = vec_p.tile([128, NB, 2], FP32, tag="Gp")
    colf = vec_p.tile([128, NB], FP32, tag="colf")
    ga = vec_p.tile([128, NB], FP32, tag="ga")
    gb = vec_p.tile([128, NB], FP32, tag="gb")

    def mv2_group(c):
        sl = slice(c * 512, (c + 1) * 512)
        g_ps = ps_tr.tile([2, 512], FP32, tag="trb")
        for a in range(NB):
            nc.tensor.matmul(
                g_ps[:],
                lhsT=Rk[:, a, :],
                rhs=m_tiles[a][:, sl],
                start=(a == 0),
                stop=(a == NB - 1),
            )
        return g_ps

    def mv2_tail(c, g_ps):
        sl = slice(c * 512, (c + 1) * 512)
        tsl = slice(c * 4, (c + 1) * 4)
        if c % 2 == 0:
            nc.vector.tensor_copy(g_sb[:, sl], g_ps[:])
        else:
            nc.scalar.copy(g_sb[:, sl], g_ps[:])
        gtp = ps_tr.tile([128, 8], FP32, tag="trb")
        for t4 in range(4):
            t = c * 4 + t4
            nc.tensor.matmul(
                gtp[:, 2 * t4 : 2 * t4 + 2],
                lhsT=g_sb[:, t * 128 : (t + 1) * 128],
                rhs=ident_f[0:2, 0:2],
                is_transpose=True, start=(t4 == 0), stop=(t4 == 3),
            )
        if c % 2 == 0:
            nc.vector.tensor_copy(Gp[:, tsl, :], gtp[:])
        else:
            nc.scalar.copy(Gp[:, tsl, :], gtp[:])
        # col = p*(G0 + c3) + q*(G1 + c4)
        nc.vector.tensor_scalar(ga[:, tsl], Gp[:, tsl, 0], c3[:, 0:1], None, ALU.add)
        nc.vector.tensor_scalar(gb[:, tsl], Gp[:, tsl, 1], c4[:, 0:1], None, ALU.add)
        nc.vector.tensor_tensor(ga[:, tsl], p_v[:, tsl], ga[:, tsl], ALU.mult)
        nc.vector.tensor_tensor(gb[:, tsl], q_v[:, tsl], gb[:, tsl], ALU.mult)
        nc.vector.tensor_tensor(colf[:, tsl], ga[:, tsl], gb[:, tsl], ALU.add)
        for t4 in range(4):
            t = c * 4 + t4
            o_sb = outsb_p.tile([128, 128], FP32, tag="o_sb")
            if t % 2 == 0:
                nc.vector.tensor_scalar(
                    o_sb[:], lrl_sb[:, t, :], colf[:, t : t + 1], None, ALU.mult
                )
            else:
                nc.scalar.activation(
                    o_sb[:], lrl_sb[:, t, :], AFT.Copy, scale=colf[:, t : t + 1]
                )
            (nc.sync if t % 2 == 0 else nc.gpsimd).dma_start(
                out_view[:, t, :], o_sb[:]
            )

    emit_lrl_transposes()
    pending = None
    for c in range(NC4):
        g_ps_c = mv2_group(c)
        if pending is not None:
            mv2_tail(pending[0], pending[1])
        pending = (c, g_ps_c)
    mv2_tail(pending[0], pending[1])


def build_nc(num_devices: int = 8) -> "bass.Bass":
    nc = bacc.Bacc(
        "TRN2", target_bir_lowering=False, debug=False, num_devices=num_devices
    )
    x_d = nc.dram_tensor("x", [N, F], FP32, kind="ExternalInput")
    W_d = nc.dram_tensor("W", [F, F], FP32, kind="ExternalInput")
    wm_d = nc.dram_tensor("w_mlp", [F], FP32, kind="ExternalInput")
    bm_d = nc.dram_tensor("b_mlp", [1], FP32, kind="ExternalInput")
    out_d = nc.dram_tensor("out", [N, F], FP32, kind="ExternalOutput")
    with tile.TileContext(nc) as tc:
        with ExitStack() as ctx:
            gat_kernel(ctx, tc, out_d.ap(), x_d.ap(), W_d.ap(), wm_d.ap(), bm_d.ap())
    nc.compile()
    return nc


_NC_CACHE: dict = {}


def run(x, W, w_mlp, b_mlp, trace=False, **spmd_kwargs):
    x = np.asarray(x, dtype=np.float32)
    W = np.asarray(W, dtype=np.float32)
    w_mlp = np.asarray(w_mlp, dtype=np.float32)
    b_mlp = np.asarray(b_mlp, dtype=np.float32)

    if "nc" not in _NC_CACHE:
        _NC_CACHE["nc"] = build_nc(num_devices=B)
    nc = _NC_CACHE["nc"]

    in_maps = [
        {"x": np.ascontiguousarray(x[b, 0]), "W": W, "w_mlp": w_mlp, "b_mlp": b_mlp}
        for b in range(B)
    ]
    res = run_bass_kernel_spmd(
        nc, in_maps, core_ids=list(range(B)), trace=trace, **spmd_kwargs
    )
    out = np.stack([res.results[b]["out"] for b in range(B)])[:, None]
    return out.astype(np.float32), res


def kernel(x, W, w_mlp, b_mlp):
    out, _ = run(x, W, w_mlp, b_mlp)
    return out
